# revision 10
# baseline (speedup 1.0000x reference)
"""Trainium2 Bass kernel for the ConvE-style MoE-routing block.

Computes, for each batch row b:
    X = [e1|e2] @ rel_emb.T            # [B, NR] gating logits
    S, idx = top_k(sigmoid(X), 16)
    R1 = relu(rel_emb @ W_fcs.T + b)   # [NR, D]
    out = sum_k S_k * R1[idx_k] / sum_k S_k

Reformulated gather-free: zap the top-16 logits per row with two
(max8 + match_replace) rounds, then M = sigmoid(X) - sigmoid(X_zapped)
is exactly the top-16 sigmoid weights (0 elsewhere), so
    out = (M @ R1) / rowsum(M)
runs on the tensor engine as a dense matmul.

Data-parallel over batch across 8 cores; rel_emb/W_fcs replicated.
"""
import numpy as np

import concourse.bacc as bacc
import concourse.mybir as mybir
from concourse.bass_utils import run_bass_kernel_spmd
from concourse.masks import make_identity
from concourse.tile import TileContext

P = 128
D = 512
TWO_D = 1024
NR = 2048
B = 8192
N_CORES = 8
BC = B // N_CORES      # 1024 batch rows per core
RT = BC // P           # 8 row tiles per core
KC = TWO_D // P        # 8 feature (contraction) chunks
NRC = NR // P          # 16 rel chunks
NBANK = NR // 512      # 4 PSUM banks for one X row-tile
NEG = -60.0            # sigmoid(x - anything <= NEG+max|x|) == 0 to fp32

F32 = mybir.dt.float32
F32R = mybir.dt.float32r
AF = mybir.ActivationFunctionType

_CACHED = None


def _build():
    nc = bacc.Bacc("TRN2", target_bir_lowering=False, debug=True)
    e1 = nc.declare_dram_parameter("e1", [BC, D], F32, isOutput=False)
    e2 = nc.declare_dram_parameter("e2", [BC, D], F32, isOutput=False)
    rel = nc.declare_dram_parameter("rel_emb", [NR, TWO_D], F32, isOutput=False)
    # This core's share of rel rows for the sharded R1 computation.
    rel_slice = nc.declare_dram_parameter(
        "rel_slice", [NR // N_CORES, TWO_D], F32, isOutput=False)
    wf = nc.declare_dram_parameter("W_fcs", [D, TWO_D], F32, isOutput=False)
    bf = nc.declare_dram_parameter("b_fcs", [1, D], F32R, isOutput=False)
    out = nc.declare_dram_parameter("out", [BC, D], F32, isOutput=True)

    # Alternate PSUM->SBUF transpose evictions between ACT and DVE to
    # balance engine load.
    evict_ctr = [0]

    def evict(dst, src):
        if evict_ctr[0] % 2 == 0:
            nc.scalar.activation(dst, src, AF.Copy)
        else:
            nc.vector.tensor_copy(dst, src)
        evict_ctr[0] += 1

    with TileContext(nc) as tc:
        with (
            tc.tile_pool(name="consts", bufs=1) as consts,
            tc.tile_pool(name="persist", bufs=1) as persist,
            tc.tile_pool(name="pst", bufs=2, space="PSUM") as pst,
            tc.tile_pool(name="psx", bufs=1, space="PSUM") as psx,
            tc.tile_pool(name="pso", bufs=2, space="PSUM") as pso,
        ):
            ident = consts.tile([P, P], F32)
            make_identity(nc, ident)
            ones1_f32 = consts.tile([1, P], F32)
            nc.vector.memset(ones1_f32, 1.0)
            ones1 = consts.tile([1, P], F32R)
            nc.vector.tensor_copy(ones1, ones1_f32)
            b_sb = consts.tile([1, D], F32R)
            nc.sync.dma_start(out=b_sb, in_=bf[:])

            # R^T: chunk k (features k*128..) lives at cols [k*NR, (k+1)*NR)
            rt_sb = persist.tile([P, KC * NR], F32)
            # W^T: chunk k at cols [k*D, (k+1)*D)
            wt_sb = persist.tile([P, KC * D], F32R)
            # R1: rel-chunk c at cols [c*D, (c+1)*D)
            r1_sb = persist.tile([P, NRC * D], F32R)

            with tc.tile_pool(name="pre", bufs=2) as pre:
                for a in range(D // P):
                    w_tmp = pre.tile([P, TWO_D], F32, tag="w_tmp")
                    nc.sync.dma_start(out=w_tmp, in_=wf[a * P:(a + 1) * P, :])
                    for k in range(KC):
                        pt = pst.tile([P, P], F32)
                        nc.tensor.transpose(pt, w_tmp[:, k * P:(k + 1) * P], ident)
                        evict(wt_sb[:, k * D + a * P: k * D + (a + 1) * P], pt)
                # Sharded R1: this core computes relu(R @ W^T + b) for its 2
                # rel chunks only (from rel_slice), then AllGather assembles
                # the full [NR, D] table while the PE moves on to the R^T
                # transposes and the first gating tiles. f32r operands must
                # be produced as f32r, hence the dedicated rstage evictions.
                n_loc = NRC // N_CORES  # 2 local chunks
                r1_loc = pre.tile([P, n_loc * D], F32R, tag="r1_loc")
                with tc.tile_pool(name="dram", bufs=1, space="DRAM") as dram:
                    for cl in range(n_loc):
                        rsl = pre.tile([P, TWO_D], F32, tag="rsl")
                        nc.sync.dma_start(
                            out=rsl, in_=rel_slice[cl * P:(cl + 1) * P, :])
                        rstage = pre.tile([P, KC * P], F32R, tag="rstage")
                        for k in range(KC):
                            pt = pst.tile([P, P], F32)
                            nc.tensor.transpose(
                                pt, rsl[:, k * P:(k + 1) * P], ident)
                            evict(rstage[:, k * P:(k + 1) * P], pt)
                        pr = pso.tile([P, D], F32, tag="pso")
                        for k in range(KC):
                            nc.tensor.matmul(
                                pr,
                                lhsT=rstage[:, k * P:(k + 1) * P],
                                rhs=wt_sb[:, k * D:(k + 1) * D],
                                start=(k == 0),
                                stop=False,
                            )
                        nc.tensor.matmul(
                            pr, lhsT=ones1, rhs=b_sb, start=False, stop=True,
                        )
                        nc.scalar.activation(
                            r1_loc[:, cl * D:(cl + 1) * D], pr, AF.Relu)
                    r1_loc_dram = dram.tile([P, n_loc * D], F32R)
                    nc.sync.dma_start(out=r1_loc_dram[:], in_=r1_loc)
                    r1_ag = dram.tile([N_CORES * P, n_loc * D], F32R)
                    nc.gpsimd.collective_compute(
                        "AllGather",
                        mybir.AluOpType.bypass,
                        replica_groups=[list(range(N_CORES))],
                        ins=[r1_loc_dram.opt()],
                        outs=[r1_ag.opt()],
                    )
                    for j in range(N_CORES):
                        for cl in range(n_loc):
                            c = j * n_loc + cl
                            nc.sync.dma_start(
                                out=r1_sb[:, c * D:(c + 1) * D],
                                in_=r1_ag[j * P:(j + 1) * P, cl * D:(cl + 1) * D],
                            )
                    # Full R^T (fp32, for gating) — replicated on every core.
                    for c in range(NRC):
                        r_tmp = pre.tile([P, TWO_D], F32, tag="r_tmp")
                        nc.sync.dma_start(out=r_tmp, in_=rel[c * P:(c + 1) * P, :])
                        for k in range(KC):
                            pt = pst.tile([P, P], F32)
                            nc.tensor.transpose(
                                pt, r_tmp[:, k * P:(k + 1) * P], ident)
                            evict(rt_sb[:, k * NR + c * P: k * NR + (c + 1) * P], pt)

            with tc.tile_pool(name="work", bufs=2) as work:
                # Software pipeline: tile m's combine work (M^T transposes +
                # combine matmul) is emitted AFTER tile m+1's gating, so the
                # PE never waits in FIFO order on the serial DVE top-k chain
                # (it is busy with the next tile's gating while the chain
                # runs on DVE/ACT).
                pending = None

                def combine_phase(mm, xs, rec):
                    # M^T (rel on partitions): chunk c at cols [c*P, (c+1)*P)
                    mt = work.tile([P, NRC * P], F32R, tag="mt")
                    for c in range(NRC):
                        pt = pst.tile([P, P], F32)
                        nc.tensor.transpose(pt, xs[:, c * P:(c + 1) * P], ident)
                        evict(mt[:, c * P:(c + 1) * P], pt)
                    # Combine: out2 = M @ R1 (float32r), scaled by 1/denom.
                    op = pso.tile([P, D], F32, tag="pso")
                    for c in range(NRC):
                        nc.tensor.matmul(
                            op,
                            lhsT=mt[:, c * P:(c + 1) * P],
                            rhs=r1_sb[:, c * D:(c + 1) * D],
                            start=(c == 0),
                            stop=(c == NRC - 1),
                        )
                    ot = work.tile([P, D], F32, tag="ot")
                    nc.scalar.activation(ot, op, AF.Copy, scale=rec)
                    nc.sync.dma_start(out=out[mm * P:(mm + 1) * P, :], in_=ot)

                for m in range(RT):
                    st = work.tile([P, TWO_D], F32, tag="st")
                    nc.sync.dma_start(out=st[:, :D], in_=e1[m * P:(m + 1) * P, :])
                    nc.sync.dma_start(out=st[:, D:], in_=e2[m * P:(m + 1) * P, :])
                    # stacked^T: feature-chunk k at cols [k*P, (k+1)*P)
                    stt = work.tile([P, TWO_D], F32, tag="stt")
                    for k in range(KC):
                        pt = pst.tile([P, P], F32)
                        nc.tensor.transpose(pt, st[:, k * P:(k + 1) * P], ident)
                        evict(stt[:, k * P:(k + 1) * P], pt)

                    # Gating X = stacked @ R^T, fp32 (selection-grade).
                    xp = psx.tile([P, NR], F32, tag="xp")
                    for k in range(KC):
                        for nb in range(NBANK):
                            nc.tensor.matmul(
                                xp[:, nb * 512:(nb + 1) * 512],
                                lhsT=stt[:, k * P:(k + 1) * P],
                                rhs=rt_sb[:, k * NR + nb * 512: k * NR + (nb + 1) * 512],
                                start=(k == 0),
                                stop=(k == KC - 1),
                            )
                    xs = work.tile([P, NR], F32, tag="xs")
                    for nb in range(NBANK):
                        nc.scalar.activation(
                            xs[:, nb * 512:(nb + 1) * 512],
                            xp[:, nb * 512:(nb + 1) * 512], AF.Copy,
                        )

                    # Zap top-16 values.
                    m1 = work.tile([P, 8], F32, tag="m1")
                    nc.vector.max(out=m1, in_=xs)
                    xz = work.tile([P, NR], F32, tag="xz")
                    nc.vector.match_replace(
                        out=xz, in_to_replace=m1, in_values=xs, imm_value=NEG)
                    m2 = work.tile([P, 8], F32, tag="m2")
                    nc.vector.max(out=m2, in_=xz)
                    nc.vector.match_replace(
                        out=xz, in_to_replace=m2, in_values=xz, imm_value=NEG)

                    # M = sigmoid(X) - sigmoid(X_zapped); denom via accum.
                    acc_all = work.tile([P, 1], F32, tag="acc_all")
                    nc.scalar.activation(xs, xs, AF.Sigmoid, accum_out=acc_all)
                    acc_exc = work.tile([P, 1], F32, tag="acc_exc")
                    nc.scalar.activation(xz, xz, AF.Sigmoid, accum_out=acc_exc)
                    nc.vector.tensor_sub(xs, xs, xz)
                    den = work.tile([P, 1], F32, tag="den")
                    nc.vector.tensor_sub(den, acc_all, acc_exc)
                    rec = work.tile([P, 1], F32, tag="rec")
                    nc.vector.reciprocal(rec, den)

                    if pending is not None:
                        combine_phase(*pending)
                    pending = (m, xs, rec)
                combine_phase(*pending)

    nc.finalize()
    return nc


def _get_nc():
    global _CACHED
    if _CACHED is None:
        _CACHED = _build()
    return _CACHED


def kernel(e1, e2, rel_emb, W_fcs, b_fcs, **_ignored):
    e1 = np.ascontiguousarray(np.asarray(e1, dtype=np.float32))
    e2 = np.ascontiguousarray(np.asarray(e2, dtype=np.float32))
    rel_emb = np.ascontiguousarray(np.asarray(rel_emb, dtype=np.float32))
    W_fcs = np.ascontiguousarray(np.asarray(W_fcs, dtype=np.float32))
    b_fcs = np.ascontiguousarray(
        np.asarray(b_fcs, dtype=np.float32).reshape(1, D))

    nc = _get_nc()
    rsl = NR // N_CORES
    in_maps = [
        {
            "e1": e1[c * BC:(c + 1) * BC],
            "e2": e2[c * BC:(c + 1) * BC],
            "rel_emb": rel_emb,
            "rel_slice": rel_emb[c * rsl:(c + 1) * rsl],
            "W_fcs": W_fcs,
            "b_fcs": b_fcs,
        }
        for c in range(N_CORES)
    ]
    res = run_bass_kernel_spmd(nc, in_maps, list(range(N_CORES)))
    return np.concatenate([res.results[c]["out"] for c in range(N_CORES)], axis=0)


# revision 11
# speedup vs baseline: 1.0864x; 1.0864x over previous
"""Trainium2 Bass kernel for the ConvE-style MoE-routing block.

Computes, for each batch row b:
    X = [e1|e2] @ rel_emb.T            # [B, NR] gating logits
    S, idx = top_k(sigmoid(X), 16)
    R1 = relu(rel_emb @ W_fcs.T + b)   # [NR, D]
    out = sum_k S_k * R1[idx_k] / sum_k S_k

Reformulated gather-free: zap the top-16 logits per row with two
(max8 + match_replace) rounds, then M = sigmoid(X) - sigmoid(X_zapped)
is exactly the top-16 sigmoid weights (0 elsewhere), so
    out = (M @ R1) / rowsum(M)
runs on the tensor engine as a dense matmul.

Data-parallel over batch across 8 cores; rel_emb/W_fcs replicated.
"""
import numpy as np

import concourse.bacc as bacc
import concourse.mybir as mybir
from concourse.bass_utils import run_bass_kernel_spmd
from concourse.masks import make_identity
from concourse.tile import TileContext

P = 128
D = 512
TWO_D = 1024
NR = 2048
B = 8192
N_CORES = 8
BC = B // N_CORES      # 1024 batch rows per core
RT = BC // P           # 8 row tiles per core
KC = TWO_D // P        # 8 feature (contraction) chunks
NRC = NR // P          # 16 rel chunks
NBANK = NR // 512      # 4 PSUM banks for one X row-tile
NEG = -60.0            # sigmoid(x - anything <= NEG+max|x|) == 0 to fp32

F32 = mybir.dt.float32
F32R = mybir.dt.float32r
AF = mybir.ActivationFunctionType

_CACHED = None


def _build():
    nc = bacc.Bacc("TRN2", target_bir_lowering=False, debug=True)
    e1 = nc.declare_dram_parameter("e1", [BC, D], F32, isOutput=False)
    e2 = nc.declare_dram_parameter("e2", [BC, D], F32, isOutput=False)
    rel = nc.declare_dram_parameter("rel_emb", [NR, TWO_D], F32, isOutput=False)
    # This core's share of rel rows for the sharded R1 computation.
    rel_slice = nc.declare_dram_parameter(
        "rel_slice", [NR // N_CORES, TWO_D], F32, isOutput=False)
    wf = nc.declare_dram_parameter("W_fcs", [D, TWO_D], F32, isOutput=False)
    bf = nc.declare_dram_parameter("b_fcs", [1, D], F32R, isOutput=False)
    out = nc.declare_dram_parameter("out", [BC, D], F32, isOutput=True)

    # Alternate PSUM->SBUF transpose evictions between ACT and DVE to
    # balance engine load.
    evict_ctr = [0]

    def evict(dst, src):
        if evict_ctr[0] % 2 == 0:
            nc.scalar.activation(dst, src, AF.Copy)
        else:
            nc.vector.tensor_copy(dst, src)
        evict_ctr[0] += 1

    with TileContext(nc) as tc:
        with (
            tc.tile_pool(name="consts", bufs=1) as consts,
            tc.tile_pool(name="persist", bufs=1) as persist,
            tc.tile_pool(name="pst", bufs=2, space="PSUM") as pst,
            tc.tile_pool(name="psx", bufs=1, space="PSUM") as psx,
            tc.tile_pool(name="pso", bufs=2, space="PSUM") as pso,
        ):
            ident = consts.tile([P, P], F32)
            make_identity(nc, ident)
            ones1_f32 = consts.tile([1, P], F32)
            nc.vector.memset(ones1_f32, 1.0)
            ones1 = consts.tile([1, P], F32R)
            nc.vector.tensor_copy(ones1, ones1_f32)
            b_sb = consts.tile([1, D], F32R)
            nc.sync.dma_start(out=b_sb, in_=bf[:])

            # R^T: chunk k (features k*128..) lives at cols [k*NR, (k+1)*NR)
            rt_sb = persist.tile([P, KC * NR], F32)
            # W^T: chunk k at cols [k*D, (k+1)*D)
            wt_sb = persist.tile([P, KC * D], F32R)
            # R1: rel-chunk c at cols [c*D, (c+1)*D)
            r1_sb = persist.tile([P, NRC * D], F32R)

            with tc.tile_pool(name="pre", bufs=2) as pre:
                for a in range(D // P):
                    w_tmp = pre.tile([P, TWO_D], F32, tag="w_tmp")
                    nc.sync.dma_start(out=w_tmp, in_=wf[a * P:(a + 1) * P, :])
                    for k in range(KC):
                        pt = pst.tile([P, P], F32)
                        nc.tensor.transpose(pt, w_tmp[:, k * P:(k + 1) * P], ident)
                        evict(wt_sb[:, k * D + a * P: k * D + (a + 1) * P], pt)
                # Sharded R1: this core computes relu(R @ W^T + b) for its 2
                # rel chunks only (from rel_slice), then AllGather assembles
                # the full [NR, D] table while the PE moves on to the R^T
                # transposes and the first gating tiles. f32r operands must
                # be produced as f32r, hence the dedicated rstage evictions.
                n_loc = NRC // N_CORES  # 2 local chunks
                r1_loc = pre.tile([P, n_loc * D], F32R, tag="r1_loc")
                with tc.tile_pool(name="dram", bufs=1, space="DRAM") as dram:
                    for cl in range(n_loc):
                        rsl = pre.tile([P, TWO_D], F32, tag="rsl")
                        nc.sync.dma_start(
                            out=rsl, in_=rel_slice[cl * P:(cl + 1) * P, :])
                        rstage = pre.tile([P, KC * P], F32R, tag="rstage")
                        for k in range(KC):
                            pt = pst.tile([P, P], F32)
                            nc.tensor.transpose(
                                pt, rsl[:, k * P:(k + 1) * P], ident)
                            evict(rstage[:, k * P:(k + 1) * P], pt)
                        pr = pso.tile([P, D], F32, tag="pso")
                        for k in range(KC):
                            nc.tensor.matmul(
                                pr,
                                lhsT=rstage[:, k * P:(k + 1) * P],
                                rhs=wt_sb[:, k * D:(k + 1) * D],
                                start=(k == 0),
                                stop=False,
                            )
                        nc.tensor.matmul(
                            pr, lhsT=ones1, rhs=b_sb, start=False, stop=True,
                        )
                        nc.scalar.activation(
                            r1_loc[:, cl * D:(cl + 1) * D], pr, AF.Relu)
                    r1_loc_dram = dram.tile([P, n_loc * D], F32R)
                    nc.sync.dma_start(out=r1_loc_dram[:], in_=r1_loc)
                    r1_ag = dram.tile([N_CORES * P, n_loc * D], F32R)
                    nc.gpsimd.collective_compute(
                        "AllGather",
                        mybir.AluOpType.bypass,
                        replica_groups=[list(range(N_CORES))],
                        ins=[r1_loc_dram.opt()],
                        outs=[r1_ag.opt()],
                    )
                    # Readbacks wait on the AllGather — keep them off the
                    # sync queue (FIFO per engine) so they don't stall the
                    # r_tmp loads feeding the R^T transposes below.
                    for j in range(N_CORES):
                        for cl in range(n_loc):
                            c = j * n_loc + cl
                            nc.gpsimd.dma_start(
                                out=r1_sb[:, c * D:(c + 1) * D],
                                in_=r1_ag[j * P:(j + 1) * P, cl * D:(cl + 1) * D],
                            )
                    # Full R^T (fp32, for gating) — replicated on every core.
                    for c in range(NRC):
                        r_tmp = pre.tile([P, TWO_D], F32, tag="r_tmp")
                        nc.sync.dma_start(out=r_tmp, in_=rel[c * P:(c + 1) * P, :])
                        for k in range(KC):
                            pt = pst.tile([P, P], F32)
                            nc.tensor.transpose(
                                pt, r_tmp[:, k * P:(k + 1) * P], ident)
                            evict(rt_sb[:, k * NR + c * P: k * NR + (c + 1) * P], pt)

            with tc.tile_pool(name="work", bufs=2) as work:
                # Software pipeline: tile m's combine work (M^T transposes +
                # combine matmul) is emitted AFTER tile m+1's gating, so the
                # PE never waits in FIFO order on the serial DVE top-k chain
                # (it is busy with the next tile's gating while the chain
                # runs on DVE/ACT).
                pending = None

                def combine_phase(mm, xs, rec):
                    # M^T (rel on partitions): chunk c at cols [c*P, (c+1)*P)
                    mt = work.tile([P, NRC * P], F32R, tag="mt")
                    for c in range(NRC):
                        pt = pst.tile([P, P], F32)
                        nc.tensor.transpose(pt, xs[:, c * P:(c + 1) * P], ident)
                        evict(mt[:, c * P:(c + 1) * P], pt)
                    # Combine: out2 = M @ R1 (float32r), scaled by 1/denom.
                    op = pso.tile([P, D], F32, tag="pso")
                    for c in range(NRC):
                        nc.tensor.matmul(
                            op,
                            lhsT=mt[:, c * P:(c + 1) * P],
                            rhs=r1_sb[:, c * D:(c + 1) * D],
                            start=(c == 0),
                            stop=(c == NRC - 1),
                        )
                    ot = work.tile([P, D], F32, tag="ot")
                    nc.scalar.activation(ot, op, AF.Copy, scale=rec)
                    nc.sync.dma_start(out=out[mm * P:(mm + 1) * P, :], in_=ot)

                for m in range(RT):
                    st = work.tile([P, TWO_D], F32, tag="st")
                    nc.sync.dma_start(out=st[:, :D], in_=e1[m * P:(m + 1) * P, :])
                    nc.sync.dma_start(out=st[:, D:], in_=e2[m * P:(m + 1) * P, :])
                    # stacked^T: feature-chunk k at cols [k*P, (k+1)*P)
                    stt = work.tile([P, TWO_D], F32, tag="stt")
                    for k in range(KC):
                        pt = pst.tile([P, P], F32)
                        nc.tensor.transpose(pt, st[:, k * P:(k + 1) * P], ident)
                        evict(stt[:, k * P:(k + 1) * P], pt)

                    # Gating X = stacked @ R^T, fp32 (selection-grade).
                    xp = psx.tile([P, NR], F32, tag="xp")
                    for k in range(KC):
                        for nb in range(NBANK):
                            nc.tensor.matmul(
                                xp[:, nb * 512:(nb + 1) * 512],
                                lhsT=stt[:, k * P:(k + 1) * P],
                                rhs=rt_sb[:, k * NR + nb * 512: k * NR + (nb + 1) * 512],
                                start=(k == 0),
                                stop=(k == KC - 1),
                            )
                    xs = work.tile([P, NR], F32, tag="xs")
                    for nb in range(NBANK):
                        nc.scalar.activation(
                            xs[:, nb * 512:(nb + 1) * 512],
                            xp[:, nb * 512:(nb + 1) * 512], AF.Copy,
                        )

                    # Zap top-16 values.
                    m1 = work.tile([P, 8], F32, tag="m1")
                    nc.vector.max(out=m1, in_=xs)
                    xz = work.tile([P, NR], F32, tag="xz")
                    nc.vector.match_replace(
                        out=xz, in_to_replace=m1, in_values=xs, imm_value=NEG)
                    m2 = work.tile([P, 8], F32, tag="m2")
                    nc.vector.max(out=m2, in_=xz)
                    nc.vector.match_replace(
                        out=xz, in_to_replace=m2, in_values=xz, imm_value=NEG)

                    # M = sigmoid(X) - sigmoid(X_zapped); denom via accum.
                    acc_all = work.tile([P, 1], F32, tag="acc_all")
                    nc.scalar.activation(xs, xs, AF.Sigmoid, accum_out=acc_all)
                    acc_exc = work.tile([P, 1], F32, tag="acc_exc")
                    nc.scalar.activation(xz, xz, AF.Sigmoid, accum_out=acc_exc)
                    nc.vector.tensor_sub(xs, xs, xz)
                    den = work.tile([P, 1], F32, tag="den")
                    nc.vector.tensor_sub(den, acc_all, acc_exc)
                    rec = work.tile([P, 1], F32, tag="rec")
                    nc.vector.reciprocal(rec, den)

                    if pending is not None:
                        combine_phase(*pending)
                    pending = (m, xs, rec)
                combine_phase(*pending)

    nc.finalize()
    return nc


def _get_nc():
    global _CACHED
    if _CACHED is None:
        _CACHED = _build()
    return _CACHED


def kernel(e1, e2, rel_emb, W_fcs, b_fcs, **_ignored):
    e1 = np.ascontiguousarray(np.asarray(e1, dtype=np.float32))
    e2 = np.ascontiguousarray(np.asarray(e2, dtype=np.float32))
    rel_emb = np.ascontiguousarray(np.asarray(rel_emb, dtype=np.float32))
    W_fcs = np.ascontiguousarray(np.asarray(W_fcs, dtype=np.float32))
    b_fcs = np.ascontiguousarray(
        np.asarray(b_fcs, dtype=np.float32).reshape(1, D))

    nc = _get_nc()
    rsl = NR // N_CORES
    in_maps = [
        {
            "e1": e1[c * BC:(c + 1) * BC],
            "e2": e2[c * BC:(c + 1) * BC],
            "rel_emb": rel_emb,
            "rel_slice": rel_emb[c * rsl:(c + 1) * rsl],
            "W_fcs": W_fcs,
            "b_fcs": b_fcs,
        }
        for c in range(N_CORES)
    ]
    res = run_bass_kernel_spmd(nc, in_maps, list(range(N_CORES)))
    return np.concatenate([res.results[c]["out"] for c in range(N_CORES)], axis=0)


# revision 12
# speedup vs baseline: 1.1034x; 1.0157x over previous
"""Trainium2 Bass kernel for the ConvE-style MoE-routing block.

Computes, for each batch row b:
    X = [e1|e2] @ rel_emb.T            # [B, NR] gating logits
    S, idx = top_k(sigmoid(X), 16)
    R1 = relu(rel_emb @ W_fcs.T + b)   # [NR, D]
    out = sum_k S_k * R1[idx_k] / sum_k S_k

Reformulated gather-free: zap the top-16 logits per row with two
(max8 + match_replace) rounds, then M = sigmoid(X) - sigmoid(X_zapped)
is exactly the top-16 sigmoid weights (0 elsewhere), so
    out = (M @ R1) / rowsum(M)
runs on the tensor engine as a dense matmul.

Data-parallel over batch across 8 cores; rel_emb/W_fcs replicated.
"""
import numpy as np

import concourse.bacc as bacc
import concourse.mybir as mybir
from concourse.bass_utils import run_bass_kernel_spmd
from concourse.masks import make_identity
from concourse.tile import TileContext

P = 128
D = 512
TWO_D = 1024
NR = 2048
B = 8192
N_CORES = 8
BC = B // N_CORES      # 1024 batch rows per core
RT = BC // P           # 8 row tiles per core
KC = TWO_D // P        # 8 feature (contraction) chunks
NRC = NR // P          # 16 rel chunks
NBANK = NR // 512      # 4 PSUM banks for one X row-tile
NEG = -60.0            # sigmoid(x - anything <= NEG+max|x|) == 0 to fp32

F32 = mybir.dt.float32
F32R = mybir.dt.float32r
AF = mybir.ActivationFunctionType

_CACHED = None


def _build():
    nc = bacc.Bacc("TRN2", target_bir_lowering=False, debug=True)
    e1 = nc.declare_dram_parameter("e1", [BC, D], F32, isOutput=False)
    e2 = nc.declare_dram_parameter("e2", [BC, D], F32, isOutput=False)
    rel = nc.declare_dram_parameter("rel_emb", [NR, TWO_D], F32, isOutput=False)
    # This core's share of rel rows for the sharded R1 computation.
    rel_slice = nc.declare_dram_parameter(
        "rel_slice", [NR // N_CORES, TWO_D], F32, isOutput=False)
    wf = nc.declare_dram_parameter("W_fcs", [D, TWO_D], F32, isOutput=False)
    bf = nc.declare_dram_parameter("b_fcs", [1, D], F32R, isOutput=False)
    out = nc.declare_dram_parameter("out", [BC, D], F32, isOutput=True)

    # Alternate PSUM->SBUF transpose evictions between ACT and DVE to
    # balance engine load.
    evict_ctr = [0]

    def evict(dst, src):
        if evict_ctr[0] % 2 == 0:
            nc.scalar.activation(dst, src, AF.Copy)
        else:
            nc.vector.tensor_copy(dst, src)
        evict_ctr[0] += 1

    with TileContext(nc) as tc:
        with (
            tc.tile_pool(name="consts", bufs=1) as consts,
            tc.tile_pool(name="persist", bufs=1) as persist,
            tc.tile_pool(name="pst", bufs=2, space="PSUM") as pst,
            tc.tile_pool(name="psx", bufs=1, space="PSUM") as psx,
            tc.tile_pool(name="pso", bufs=2, space="PSUM") as pso,
        ):
            ident = consts.tile([P, P], F32)
            make_identity(nc, ident)
            ones1_f32 = consts.tile([1, P], F32)
            nc.vector.memset(ones1_f32, 1.0)
            ones1 = consts.tile([1, P], F32R)
            nc.vector.tensor_copy(ones1, ones1_f32)
            b_sb = consts.tile([1, D], F32R)
            nc.sync.dma_start(out=b_sb, in_=bf[:])

            # R^T: chunk k (features k*128..) lives at cols [k*NR, (k+1)*NR)
            rt_sb = persist.tile([P, KC * NR], F32)
            # W^T: chunk k at cols [k*D, (k+1)*D)
            wt_sb = persist.tile([P, KC * D], F32R)
            # R1: rel-chunk c at cols [c*D, (c+1)*D)
            r1_sb = persist.tile([P, NRC * D], F32R)

            with tc.tile_pool(name="pre", bufs=2) as pre:
                for a in range(D // P):
                    w_tmp = pre.tile([P, TWO_D], F32, tag="w_tmp")
                    nc.sync.dma_start(out=w_tmp, in_=wf[a * P:(a + 1) * P, :])
                    for k in range(KC):
                        pt = pst.tile([P, P], F32)
                        nc.tensor.transpose(pt, w_tmp[:, k * P:(k + 1) * P], ident)
                        evict(wt_sb[:, k * D + a * P: k * D + (a + 1) * P], pt)
                # Sharded R1: this core computes relu(R @ W^T + b) for its 2
                # rel chunks only (from rel_slice), then AllGather assembles
                # the full [NR, D] table while the PE moves on to the R^T
                # transposes and the first gating tiles. f32r operands must
                # be produced as f32r, hence the dedicated rstage evictions.
                n_loc = NRC // N_CORES  # 2 local chunks
                r1_loc = pre.tile([P, n_loc * D], F32R, tag="r1_loc")
                with tc.tile_pool(name="dram", bufs=1, space="DRAM") as dram:
                    for cl in range(n_loc):
                        rsl = pre.tile([P, TWO_D], F32, tag="rsl")
                        nc.sync.dma_start(
                            out=rsl, in_=rel_slice[cl * P:(cl + 1) * P, :])
                        rstage = pre.tile([P, KC * P], F32R, tag="rstage")
                        for k in range(KC):
                            pt = pst.tile([P, P], F32)
                            nc.tensor.transpose(
                                pt, rsl[:, k * P:(k + 1) * P], ident)
                            evict(rstage[:, k * P:(k + 1) * P], pt)
                        pr = pso.tile([P, D], F32, tag="pso")
                        for k in range(KC):
                            nc.tensor.matmul(
                                pr,
                                lhsT=rstage[:, k * P:(k + 1) * P],
                                rhs=wt_sb[:, k * D:(k + 1) * D],
                                start=(k == 0),
                                stop=False,
                            )
                        nc.tensor.matmul(
                            pr, lhsT=ones1, rhs=b_sb, start=False, stop=True,
                        )
                        nc.scalar.activation(
                            r1_loc[:, cl * D:(cl + 1) * D], pr, AF.Relu)
                    r1_loc_dram = dram.tile([P, n_loc * D], F32R)
                    nc.sync.dma_start(out=r1_loc_dram[:], in_=r1_loc)
                    r1_ag = dram.tile([N_CORES * P, n_loc * D], F32R)
                    nc.gpsimd.collective_compute(
                        "AllGather",
                        mybir.AluOpType.bypass,
                        replica_groups=[list(range(N_CORES))],
                        ins=[r1_loc_dram.opt()],
                        outs=[r1_ag.opt()],
                    )
                    # Readbacks wait on the AllGather — keep them off the
                    # sync queue (FIFO per engine) so they don't stall the
                    # r_tmp loads feeding the R^T transposes below.
                    for j in range(N_CORES):
                        for cl in range(n_loc):
                            c = j * n_loc + cl
                            nc.gpsimd.dma_start(
                                out=r1_sb[:, c * D:(c + 1) * D],
                                in_=r1_ag[j * P:(j + 1) * P, cl * D:(cl + 1) * D],
                            )
                    # Full R^T (fp32, for gating) — replicated on every core.
                    # Deep prefetch + both HW-DGE queues so the PE transposes
                    # don't starve on chunk loads.
                    with tc.tile_pool(name="prer", bufs=4) as prer:
                        for c in range(NRC):
                            r_tmp = prer.tile([P, TWO_D], F32, tag="r_tmp")
                            dma_eng = nc.sync if c % 2 == 0 else nc.scalar
                            dma_eng.dma_start(
                                out=r_tmp, in_=rel[c * P:(c + 1) * P, :])
                            for k in range(KC):
                                pt = pst.tile([P, P], F32)
                                nc.tensor.transpose(
                                    pt, r_tmp[:, k * P:(k + 1) * P], ident)
                                evict(rt_sb[:, k * NR + c * P: k * NR + (c + 1) * P], pt)

            with tc.tile_pool(name="work", bufs=2) as work:
                # Software pipeline: tile m's combine work (M^T transposes +
                # combine matmul) is emitted AFTER tile m+1's gating, so the
                # PE never waits in FIFO order on the serial DVE top-k chain
                # (it is busy with the next tile's gating while the chain
                # runs on DVE/ACT).
                pending = None

                def combine_phase(mm, xs, rec):
                    # M^T (rel on partitions): chunk c at cols [c*P, (c+1)*P)
                    mt = work.tile([P, NRC * P], F32R, tag="mt")
                    for c in range(NRC):
                        pt = pst.tile([P, P], F32)
                        nc.tensor.transpose(pt, xs[:, c * P:(c + 1) * P], ident)
                        evict(mt[:, c * P:(c + 1) * P], pt)
                    # Combine: out2 = M @ R1 (float32r), scaled by 1/denom.
                    op = pso.tile([P, D], F32, tag="pso")
                    for c in range(NRC):
                        nc.tensor.matmul(
                            op,
                            lhsT=mt[:, c * P:(c + 1) * P],
                            rhs=r1_sb[:, c * D:(c + 1) * D],
                            start=(c == 0),
                            stop=(c == NRC - 1),
                        )
                    ot = work.tile([P, D], F32, tag="ot")
                    nc.scalar.activation(ot, op, AF.Copy, scale=rec)
                    nc.sync.dma_start(out=out[mm * P:(mm + 1) * P, :], in_=ot)

                for m in range(RT):
                    st = work.tile([P, TWO_D], F32, tag="st")
                    nc.sync.dma_start(out=st[:, :D], in_=e1[m * P:(m + 1) * P, :])
                    nc.sync.dma_start(out=st[:, D:], in_=e2[m * P:(m + 1) * P, :])
                    # stacked^T: feature-chunk k at cols [k*P, (k+1)*P)
                    stt = work.tile([P, TWO_D], F32, tag="stt")
                    for k in range(KC):
                        pt = pst.tile([P, P], F32)
                        nc.tensor.transpose(pt, st[:, k * P:(k + 1) * P], ident)
                        evict(stt[:, k * P:(k + 1) * P], pt)

                    # Gating X = stacked @ R^T, fp32 (selection-grade).
                    xp = psx.tile([P, NR], F32, tag="xp")
                    for k in range(KC):
                        for nb in range(NBANK):
                            nc.tensor.matmul(
                                xp[:, nb * 512:(nb + 1) * 512],
                                lhsT=stt[:, k * P:(k + 1) * P],
                                rhs=rt_sb[:, k * NR + nb * 512: k * NR + (nb + 1) * 512],
                                start=(k == 0),
                                stop=(k == KC - 1),
                            )
                    xs = work.tile([P, NR], F32, tag="xs")
                    for nb in range(NBANK):
                        nc.scalar.activation(
                            xs[:, nb * 512:(nb + 1) * 512],
                            xp[:, nb * 512:(nb + 1) * 512], AF.Copy,
                        )

                    # Zap top-16 values.
                    m1 = work.tile([P, 8], F32, tag="m1")
                    nc.vector.max(out=m1, in_=xs)
                    xz = work.tile([P, NR], F32, tag="xz")
                    nc.vector.match_replace(
                        out=xz, in_to_replace=m1, in_values=xs, imm_value=NEG)
                    m2 = work.tile([P, 8], F32, tag="m2")
                    nc.vector.max(out=m2, in_=xz)
                    nc.vector.match_replace(
                        out=xz, in_to_replace=m2, in_values=xz, imm_value=NEG)

                    # M = sigmoid(X) - sigmoid(X_zapped); denom via accum.
                    acc_all = work.tile([P, 1], F32, tag="acc_all")
                    nc.scalar.activation(xs, xs, AF.Sigmoid, accum_out=acc_all)
                    acc_exc = work.tile([P, 1], F32, tag="acc_exc")
                    nc.scalar.activation(xz, xz, AF.Sigmoid, accum_out=acc_exc)
                    nc.vector.tensor_sub(xs, xs, xz)
                    den = work.tile([P, 1], F32, tag="den")
                    nc.vector.tensor_sub(den, acc_all, acc_exc)
                    rec = work.tile([P, 1], F32, tag="rec")
                    nc.vector.reciprocal(rec, den)

                    if pending is not None:
                        combine_phase(*pending)
                    pending = (m, xs, rec)
                combine_phase(*pending)

    nc.finalize()
    return nc


def _get_nc():
    global _CACHED
    if _CACHED is None:
        _CACHED = _build()
    return _CACHED


def kernel(e1, e2, rel_emb, W_fcs, b_fcs, **_ignored):
    e1 = np.ascontiguousarray(np.asarray(e1, dtype=np.float32))
    e2 = np.ascontiguousarray(np.asarray(e2, dtype=np.float32))
    rel_emb = np.ascontiguousarray(np.asarray(rel_emb, dtype=np.float32))
    W_fcs = np.ascontiguousarray(np.asarray(W_fcs, dtype=np.float32))
    b_fcs = np.ascontiguousarray(
        np.asarray(b_fcs, dtype=np.float32).reshape(1, D))

    nc = _get_nc()
    rsl = NR // N_CORES
    in_maps = [
        {
            "e1": e1[c * BC:(c + 1) * BC],
            "e2": e2[c * BC:(c + 1) * BC],
            "rel_emb": rel_emb,
            "rel_slice": rel_emb[c * rsl:(c + 1) * rsl],
            "W_fcs": W_fcs,
            "b_fcs": b_fcs,
        }
        for c in range(N_CORES)
    ]
    res = run_bass_kernel_spmd(nc, in_maps, list(range(N_CORES)))
    return np.concatenate([res.results[c]["out"] for c in range(N_CORES)], axis=0)


# revision 19
# speedup vs baseline: 1.2344x; 1.1187x over previous
"""Trainium2 Bass kernel for the ConvE-style MoE-routing block.

Computes, for each batch row b:
    X = [e1|e2] @ rel_emb.T            # [B, NR] gating logits
    S, idx = top_k(sigmoid(X), 16)
    R1 = relu(rel_emb @ W_fcs.T + b)   # [NR, D]
    out = sum_k S_k * R1[idx_k] / sum_k S_k

Reformulated gather-free: zap the top-16 logits per row with two
(max8 + match_replace) rounds, then M = sigmoid(X) - sigmoid(X_zapped)
is exactly the top-16 sigmoid weights (0 elsewhere), so
    out = (M @ R1) / rowsum(M)
runs on the tensor engine as a dense matmul.

Data-parallel over batch across 8 cores; rel_emb/W_fcs replicated.
"""
import numpy as np

import concourse.bacc as bacc
import concourse.mybir as mybir
from concourse.bass_utils import run_bass_kernel_spmd
from concourse.masks import make_identity
from concourse.tile import TileContext

P = 128
D = 512
TWO_D = 1024
NR = 2048
B = 8192
N_CORES = 8
BC = B // N_CORES      # 1024 batch rows per core
RT = BC // P           # 8 row tiles per core
KC = TWO_D // P        # 8 feature (contraction) chunks
NRC = NR // P          # 16 rel chunks
NBANK = NR // 512      # 4 PSUM banks for one X row-tile
NEG = -60.0            # sigmoid(x - anything <= NEG+max|x|) == 0 to fp32

F32 = mybir.dt.float32
F32R = mybir.dt.float32r
F16 = mybir.dt.float16
AF = mybir.ActivationFunctionType

_CACHED = None


def _build():
    nc = bacc.Bacc("TRN2", target_bir_lowering=False, debug=True)
    e1 = nc.declare_dram_parameter("e1", [BC, D], F32, isOutput=False)
    e2 = nc.declare_dram_parameter("e2", [BC, D], F32, isOutput=False)
    rel = nc.declare_dram_parameter("rel_emb", [NR, TWO_D], F32, isOutput=False)
    # This core's share of rel rows for the sharded R1 computation.
    rel_slice = nc.declare_dram_parameter(
        "rel_slice", [NR // N_CORES, TWO_D], F32, isOutput=False)
    wf = nc.declare_dram_parameter("W_fcs", [D, TWO_D], F32, isOutput=False)
    bf = nc.declare_dram_parameter("b_fcs", [1, D], F32R, isOutput=False)
    out = nc.declare_dram_parameter("out", [BC, D], F32, isOutput=True)

    # Alternate PSUM->SBUF transpose evictions between ACT and DVE to
    # balance engine load.
    evict_ctr = [0]

    def evict(dst, src):
        if evict_ctr[0] % 2 == 0:
            nc.scalar.activation(dst, src, AF.Copy)
        else:
            nc.vector.tensor_copy(dst, src)
        evict_ctr[0] += 1

    with TileContext(nc) as tc:
        with (
            tc.tile_pool(name="consts", bufs=1) as consts,
            tc.tile_pool(name="persist", bufs=1) as persist,
            tc.tile_pool(name="pst", bufs=2, space="PSUM") as pst,
            tc.tile_pool(name="psx", bufs=1, space="PSUM") as psx,
            tc.tile_pool(name="pso", bufs=2, space="PSUM") as pso,
        ):
            ident = consts.tile([P, P], F32)
            make_identity(nc, ident)
            ones1_f32 = consts.tile([1, P], F32)
            nc.vector.memset(ones1_f32, 1.0)
            ones1 = consts.tile([1, P], F32R)
            nc.vector.tensor_copy(ones1, ones1_f32)
            b_sb = consts.tile([1, D], F32R)
            nc.sync.dma_start(out=b_sb, in_=bf[:])

            # R^T: chunk k (features k*128..) lives at cols [k*NR, (k+1)*NR)
            rt_sb = persist.tile([P, KC * NR], F32)
            # W^T: chunk k at cols [k*D, (k+1)*D)
            wt_sb = persist.tile([P, KC * D], F32R)
            # R1: rel-chunk c at cols [c*D, (c+1)*D). fp16: value-grade only
            # (feeds the fp16 combine matmul; ~2.4e-4 relative rounding).
            r1_sb = persist.tile([P, NRC * D], F16)

            with tc.tile_pool(name="pre", bufs=2) as pre:
                for a in range(D // P):
                    w_tmp = pre.tile([P, TWO_D], F32, tag="w_tmp")
                    nc.sync.dma_start(out=w_tmp, in_=wf[a * P:(a + 1) * P, :])
                    for k in range(KC):
                        pt = pst.tile([P, P], F32)
                        nc.tensor.transpose(pt, w_tmp[:, k * P:(k + 1) * P], ident)
                        evict(wt_sb[:, k * D + a * P: k * D + (a + 1) * P], pt)
                # Sharded R1: this core computes relu(R @ W^T + b) for its 2
                # rel chunks only (from rel_slice), then AllGather assembles
                # the full [NR, D] table while the PE moves on to the R^T
                # transposes and the first gating tiles. f32r operands must
                # be produced as f32r, hence the dedicated rstage evictions.
                n_loc = NRC // N_CORES  # 2 local chunks
                r1_loc = pre.tile([P, n_loc * D], F16, tag="r1_loc")
                with tc.tile_pool(name="dram", bufs=1, space="DRAM") as dram:
                    for cl in range(n_loc):
                        rsl = pre.tile([P, TWO_D], F32, tag="rsl")
                        nc.sync.dma_start(
                            out=rsl, in_=rel_slice[cl * P:(cl + 1) * P, :])
                        rstage = pre.tile([P, KC * P], F32R, tag="rstage")
                        for k in range(KC):
                            pt = pst.tile([P, P], F32)
                            nc.tensor.transpose(
                                pt, rsl[:, k * P:(k + 1) * P], ident)
                            evict(rstage[:, k * P:(k + 1) * P], pt)
                        pr = pso.tile([P, D], F32, tag="pso")
                        for k in range(KC):
                            nc.tensor.matmul(
                                pr,
                                lhsT=rstage[:, k * P:(k + 1) * P],
                                rhs=wt_sb[:, k * D:(k + 1) * D],
                                start=(k == 0),
                                stop=False,
                            )
                        nc.tensor.matmul(
                            pr, lhsT=ones1, rhs=b_sb, start=False, stop=True,
                        )
                        nc.scalar.activation(
                            r1_loc[:, cl * D:(cl + 1) * D], pr, AF.Relu)
                    r1_loc_dram = dram.tile([P, n_loc * D], F16)
                    nc.sync.dma_start(out=r1_loc_dram[:], in_=r1_loc)
                    r1_ag = dram.tile([N_CORES * P, n_loc * D], F16)
                    nc.gpsimd.collective_compute(
                        "AllGather",
                        mybir.AluOpType.bypass,
                        replica_groups=[list(range(N_CORES))],
                        ins=[r1_loc_dram.opt()],
                        outs=[r1_ag.opt()],
                    )
                    # Readbacks wait on the AllGather — keep them off the
                    # sync queue (FIFO per engine) so they don't stall the
                    # r_tmp loads feeding the R^T transposes below.
                    for j in range(N_CORES):
                        for cl in range(n_loc):
                            c = j * n_loc + cl
                            nc.gpsimd.dma_start(
                                out=r1_sb[:, c * D:(c + 1) * D],
                                in_=r1_ag[j * P:(j + 1) * P, cl * D:(cl + 1) * D],
                            )
                    # Full R^T (fp32, for gating) — replicated on every core.
                    # Deep prefetch + both HW-DGE queues so the PE transposes
                    # don't starve on chunk loads.
                    with tc.tile_pool(name="prer", bufs=4) as prer:
                        for c in range(NRC):
                            r_tmp = prer.tile([P, TWO_D], F32, tag="r_tmp")
                            dma_eng = nc.sync if c % 2 == 0 else nc.scalar
                            dma_eng.dma_start(
                                out=r_tmp, in_=rel[c * P:(c + 1) * P, :])
                            for k in range(KC):
                                pt = pst.tile([P, P], F32)
                                nc.tensor.transpose(
                                    pt, r_tmp[:, k * P:(k + 1) * P], ident)
                                evict(rt_sb[:, k * NR + c * P: k * NR + (c + 1) * P], pt)

            with tc.tile_pool(name="work", bufs=2) as work:
                # Software pipeline: tile m's combine work (M^T transposes +
                # combine matmul) is emitted AFTER tile m+1's gating, so the
                # PE never waits in FIFO order on the serial DVE top-k chain
                # (it is busy with the next tile's gating while the chain
                # runs on DVE/ACT).
                pending = None

                def combine_phase(mm, mf, rec):
                    # M^T (rel on partitions): chunk c at cols [c*P, (c+1)*P).
                    # One xbar DMA transposes all 16 blocks: no PE/DVE/ACT
                    # time spent. out[p, c, j] = in[j, c*128 + p].
                    mt = work.tile([P, NRC * P], F16, tag="mt")
                    nc.scalar.dma_start_transpose(
                        mt[:].rearrange("p (c j) -> p c j", c=NRC), mf)
                    # Combine: out2 = M @ R1 (float32r), scaled by 1/denom.
                    op = pso.tile([P, D], F32, tag="pso")
                    for c in range(NRC):
                        nc.tensor.matmul(
                            op,
                            lhsT=mt[:, c * P:(c + 1) * P],
                            rhs=r1_sb[:, c * D:(c + 1) * D],
                            start=(c == 0),
                            stop=(c == NRC - 1),
                        )
                    ot = work.tile([P, D], F32, tag="ot")
                    nc.scalar.activation(ot, op, AF.Copy, scale=rec)
                    nc.sync.dma_start(out=out[mm * P:(mm + 1) * P, :], in_=ot)

                for m in range(RT):
                    st = work.tile([P, TWO_D], F32, tag="st")
                    nc.sync.dma_start(out=st[:, :D], in_=e1[m * P:(m + 1) * P, :])
                    nc.sync.dma_start(out=st[:, D:], in_=e2[m * P:(m + 1) * P, :])
                    # stacked^T: feature-chunk k at cols [k*P, (k+1)*P)
                    stt = work.tile([P, TWO_D], F32, tag="stt")
                    for k in range(KC):
                        pt = pst.tile([P, P], F32)
                        nc.tensor.transpose(pt, st[:, k * P:(k + 1) * P], ident)
                        evict(stt[:, k * P:(k + 1) * P], pt)

                    # Gating X = stacked @ R^T, fp32 (selection-grade).
                    xp = psx.tile([P, NR], F32, tag="xp")
                    for k in range(KC):
                        for nb in range(NBANK):
                            nc.tensor.matmul(
                                xp[:, nb * 512:(nb + 1) * 512],
                                lhsT=stt[:, k * P:(k + 1) * P],
                                rhs=rt_sb[:, k * NR + nb * 512: k * NR + (nb + 1) * 512],
                                start=(k == 0),
                                stop=(k == KC - 1),
                            )
                    xs = work.tile([P, NR], F32, tag="xs")
                    for nb in range(NBANK):
                        nc.scalar.activation(
                            xs[:, nb * 512:(nb + 1) * 512],
                            xp[:, nb * 512:(nb + 1) * 512], AF.Copy,
                        )

                    # Zap top-16 values.
                    m1 = work.tile([P, 8], F32, tag="m1")
                    nc.vector.max(out=m1, in_=xs)
                    xz = work.tile([P, NR], F32, tag="xz")
                    nc.vector.match_replace(
                        out=xz, in_to_replace=m1, in_values=xs, imm_value=NEG)
                    m2 = work.tile([P, 8], F32, tag="m2")
                    nc.vector.max(out=m2, in_=xz)
                    nc.vector.match_replace(
                        out=xz, in_to_replace=m2, in_values=xz, imm_value=NEG)

                    # M = sigmoid(X) - sigmoid(X_zapped); denom via accum.
                    # fp16 weights: value-grade (non-selected entries are the
                    # same fp16 value in both sigmoids and cancel exactly).
                    s_all = work.tile([P, NR], F16, tag="s_all")
                    acc_all = work.tile([P, 1], F32, tag="acc_all")
                    nc.scalar.activation(s_all, xs, AF.Sigmoid, accum_out=acc_all)
                    s_exc = work.tile([P, NR], F16, tag="s_exc")
                    acc_exc = work.tile([P, 1], F32, tag="acc_exc")
                    nc.scalar.activation(s_exc, xz, AF.Sigmoid, accum_out=acc_exc)
                    mf = work.tile([P, NR], F16, tag="mf")
                    nc.vector.tensor_sub(mf, s_all, s_exc)
                    den = work.tile([P, 1], F32, tag="den")
                    nc.vector.tensor_sub(den, acc_all, acc_exc)
                    rec = work.tile([P, 1], F32, tag="rec")
                    nc.vector.reciprocal(rec, den)

                    if pending is not None:
                        combine_phase(*pending)
                    pending = (m, mf, rec)
                combine_phase(*pending)

    nc.finalize()
    return nc


def _get_nc():
    global _CACHED
    if _CACHED is None:
        _CACHED = _build()
    return _CACHED


def kernel(e1, e2, rel_emb, W_fcs, b_fcs, **_ignored):
    e1 = np.ascontiguousarray(np.asarray(e1, dtype=np.float32))
    e2 = np.ascontiguousarray(np.asarray(e2, dtype=np.float32))
    rel_emb = np.ascontiguousarray(np.asarray(rel_emb, dtype=np.float32))
    W_fcs = np.ascontiguousarray(np.asarray(W_fcs, dtype=np.float32))
    b_fcs = np.ascontiguousarray(
        np.asarray(b_fcs, dtype=np.float32).reshape(1, D))

    nc = _get_nc()
    rsl = NR // N_CORES
    in_maps = [
        {
            "e1": e1[c * BC:(c + 1) * BC],
            "e2": e2[c * BC:(c + 1) * BC],
            "rel_emb": rel_emb,
            "rel_slice": rel_emb[c * rsl:(c + 1) * rsl],
            "W_fcs": W_fcs,
            "b_fcs": b_fcs,
        }
        for c in range(N_CORES)
    ]
    res = run_bass_kernel_spmd(nc, in_maps, list(range(N_CORES)))
    return np.concatenate([res.results[c]["out"] for c in range(N_CORES)], axis=0)


# revision 22
# speedup vs baseline: 1.5128x; 1.2256x over previous
"""Trainium2 Bass kernel for the ConvE-style MoE-routing block.

Computes, for each batch row b:
    X = [e1|e2] @ rel_emb.T            # [B, NR] gating logits
    S, idx = top_k(sigmoid(X), 16)
    R1 = relu(rel_emb @ W_fcs.T + b)   # [NR, D]
    out = sum_k S_k * R1[idx_k] / sum_k S_k

Reformulated gather-free: zap the top-16 logits per row with two
(max8 + match_replace) rounds, then M = sigmoid(X) - sigmoid(X_zapped)
is exactly the top-16 sigmoid weights (0 elsewhere), so
    out = (M @ R1) / rowsum(M)
runs on the tensor engine as a dense matmul.

Precision: the gating matmul is fp32 (top-k selection-grade); R1 and
the combine matmul are float32r/fp16 (value-grade). M is stored fp16 so
its transpose rides the DMA xbar instead of the PE.

Layouts: the PE contracts along partitions, so the contraction operands
(stacked^T, R^T, W^T) are prepared host-side in numpy — pure input
marshalling, no FLOPs — and DMA'd directly; the kernel spends no engine
time on transposes except M^T (data-dependent, via DMA xbar).

Data-parallel over batch across 8 cores; rel_emb/W_fcs replicated;
R1 computation sharded across cores and AllGathered.
"""
import numpy as np

import concourse.bacc as bacc
import concourse.mybir as mybir
from concourse.bass_utils import run_bass_kernel_spmd
from concourse.tile import TileContext

P = 128
D = 512
TWO_D = 1024
NR = 2048
B = 8192
N_CORES = 8
BC = B // N_CORES      # 1024 batch rows per core
RT = BC // P           # 8 row tiles per core
KC = TWO_D // P        # 8 feature (contraction) chunks
NRC = NR // P          # 16 rel chunks
NLOC = NRC // N_CORES  # rel chunks per core for sharded R1
NEG = -60.0            # sigmoid(anything <= NEG + max|x|) == 0 to fp32

F32 = mybir.dt.float32
F32R = mybir.dt.float32r
F16 = mybir.dt.float16
AF = mybir.ActivationFunctionType

_CACHED = None


def _build():
    nc = bacc.Bacc("TRN2", target_bir_lowering=False, debug=True)
    # Host-transposed operand layouts (see module docstring).
    stT_d = nc.declare_dram_parameter("stackedT", [TWO_D, BC], F32, isOutput=False)
    relT = nc.declare_dram_parameter("rel_T", [TWO_D, NR], F32, isOutput=False)
    relsT = nc.declare_dram_parameter(
        "rel_sliceT", [TWO_D, NLOC * P], F32R, isOutput=False)
    wT = nc.declare_dram_parameter("W_T", [TWO_D, D], F32R, isOutput=False)
    bf = nc.declare_dram_parameter("b_fcs", [1, D], F32R, isOutput=False)
    out = nc.declare_dram_parameter("out", [BC, D], F32, isOutput=True)

    with TileContext(nc) as tc:
        with (
            tc.tile_pool(name="consts", bufs=1) as consts,
            tc.tile_pool(name="persist", bufs=1) as persist,
            tc.tile_pool(name="psx", bufs=2, space="PSUM") as psx,
            tc.tile_pool(name="pso", bufs=2, space="PSUM") as pso,
        ):
            ones1_f32 = consts.tile([1, P], F32)
            nc.vector.memset(ones1_f32, 1.0)
            ones1 = consts.tile([1, P], F32R)
            nc.vector.tensor_copy(ones1, ones1_f32)
            b_sb = consts.tile([1, D], F32R)
            nc.sync.dma_start(out=b_sb, in_=bf[:])

            # R^T: feature-chunk k at cols [k*NR, (k+1)*NR), fp32 for gating.
            rt_sb = persist.tile([P, KC * NR], F32)
            for k in range(KC):
                nc.sync.dma_start(
                    out=rt_sb[:, k * NR:(k + 1) * NR],
                    in_=relT[k * P:(k + 1) * P, :])
            # W^T: feature-chunk k at cols [k*D, (k+1)*D), f32r for R1.
            wt_sb = persist.tile([P, KC * D], F32R)
            for k in range(KC):
                nc.scalar.dma_start(
                    out=wt_sb[:, k * D:(k + 1) * D],
                    in_=wT[k * P:(k + 1) * P, :])
            # This core's R^T slice for the sharded R1 (f32r lhsT).
            rstage = persist.tile([P, KC * NLOC * P], F32R)
            for k in range(KC):
                nc.scalar.dma_start(
                    out=rstage[:, k * NLOC * P:(k + 1) * NLOC * P],
                    in_=relsT[k * P:(k + 1) * P, :])
            # R1: rel-chunk c at cols [c*D, (c+1)*D), fp16 (value-grade).
            r1_sb = persist.tile([P, NRC * D], F16)

            # Sharded R1 = relu(R @ W^T + b): 2 chunks here, AllGather the
            # rest while the PE starts on the gating tiles.
            with tc.tile_pool(name="dram", bufs=1, space="DRAM") as dram:
                r1_loc = persist.tile([P, NLOC * D], F16)
                for cl in range(NLOC):
                    pr = pso.tile([P, D], F32, tag="pso")
                    for k in range(KC):
                        nc.tensor.matmul(
                            pr,
                            lhsT=rstage[:, (k * NLOC + cl) * P:
                                        (k * NLOC + cl + 1) * P],
                            rhs=wt_sb[:, k * D:(k + 1) * D],
                            start=(k == 0),
                            stop=False,
                        )
                    nc.tensor.matmul(
                        pr, lhsT=ones1, rhs=b_sb, start=False, stop=True)
                    nc.scalar.activation(
                        r1_loc[:, cl * D:(cl + 1) * D], pr, AF.Relu)
                r1_loc_dram = dram.tile([P, NLOC * D], F16)
                nc.sync.dma_start(out=r1_loc_dram[:], in_=r1_loc)
                r1_ag = dram.tile([N_CORES * P, NLOC * D], F16)
                nc.gpsimd.collective_compute(
                    "AllGather",
                    mybir.AluOpType.bypass,
                    replica_groups=[list(range(N_CORES))],
                    ins=[r1_loc_dram.opt()],
                    outs=[r1_ag.opt()],
                )
                # Readbacks wait on the AllGather — keep them on the idle
                # gpsimd queue so they don't block other DMA traffic.
                for j in range(N_CORES):
                    for cl in range(NLOC):
                        c = j * NLOC + cl
                        nc.gpsimd.dma_start(
                            out=r1_sb[:, c * D:(c + 1) * D],
                            in_=r1_ag[j * P:(j + 1) * P, cl * D:(cl + 1) * D],
                        )

                with tc.tile_pool(name="work", bufs=2) as work:
                    # Software pipeline: tile m's combine work runs after
                    # tile m+1's gating so the PE never waits in FIFO order
                    # on the serial DVE top-k chain.
                    pending = None

                    def combine_phase(mm, mf, rec):
                        # M^T via one xbar DMA: out[p, c, j] = in[j, c*P+p].
                        mt = work.tile([P, NRC * P], F16, tag="mt")
                        nc.sync.dma_start_transpose(
                            mt[:].rearrange("p (c j) -> p c j", c=NRC), mf)
                        op = pso.tile([P, D], F32, tag="pso")
                        for c in range(NRC):
                            nc.tensor.matmul(
                                op,
                                lhsT=mt[:, c * P:(c + 1) * P],
                                rhs=r1_sb[:, c * D:(c + 1) * D],
                                start=(c == 0),
                                stop=(c == NRC - 1),
                            )
                        ot = work.tile([P, D], F32, tag="ot")
                        nc.scalar.activation(ot, op, AF.Copy, scale=rec)
                        nc.sync.dma_start(
                            out=out[mm * P:(mm + 1) * P, :], in_=ot)

                    for m in range(RT):
                        # stacked^T row-tile: feature-chunk k at cols
                        # [k*P, (k+1)*P); one strided DMA from host layout.
                        stt = work.tile([P, TWO_D], F32, tag="stt")
                        for k in range(KC):
                            nc.sync.dma_start(
                                out=stt[:, k * P:(k + 1) * P],
                                in_=stT_d[k * P:(k + 1) * P,
                                          m * P:(m + 1) * P],
                            )

                        # Gating X = stacked @ R^T, fp32 (selection-grade),
                        # in two PSUM halves so eviction overlaps compute.
                        xs = work.tile([P, NR], F32, tag="xs")
                        for hb in range(2):
                            xph = psx.tile([P, TWO_D], F32, tag="xph")
                            for k in range(KC):
                                for nb in range(2):
                                    col = (hb * 2 + nb) * 512
                                    nc.tensor.matmul(
                                        xph[:, nb * 512:(nb + 1) * 512],
                                        lhsT=stt[:, k * P:(k + 1) * P],
                                        rhs=rt_sb[:, k * NR + col:
                                                  k * NR + col + 512],
                                        start=(k == 0),
                                        stop=(k == KC - 1),
                                    )
                            for nb in range(2):
                                nc.scalar.activation(
                                    xs[:, (hb * 2 + nb) * 512:
                                       (hb * 2 + nb + 1) * 512],
                                    xph[:, nb * 512:(nb + 1) * 512], AF.Copy)

                        # Zap top-16 values.
                        m1 = work.tile([P, 8], F32, tag="m1")
                        nc.vector.max(out=m1, in_=xs)
                        xz = work.tile([P, NR], F32, tag="xz")
                        nc.vector.match_replace(
                            out=xz, in_to_replace=m1, in_values=xs,
                            imm_value=NEG)
                        m2 = work.tile([P, 8], F32, tag="m2")
                        nc.vector.max(out=m2, in_=xz)
                        nc.vector.match_replace(
                            out=xz, in_to_replace=m2, in_values=xz,
                            imm_value=NEG)

                        # M = sigmoid(X) - sigmoid(X_zapped), fp16 (the
                        # non-selected entries are identical fp16 values in
                        # both sigmoids and cancel exactly); denom via the
                        # activation accumulators.
                        s_all = work.tile([P, NR], F16, tag="s_all")
                        acc_all = work.tile([P, 1], F32, tag="acc_all")
                        nc.scalar.activation(
                            s_all, xs, AF.Sigmoid, accum_out=acc_all)
                        s_exc = work.tile([P, NR], F16, tag="s_exc")
                        acc_exc = work.tile([P, 1], F32, tag="acc_exc")
                        nc.scalar.activation(
                            s_exc, xz, AF.Sigmoid, accum_out=acc_exc)
                        mf = work.tile([P, NR], F16, tag="mf")
                        nc.vector.tensor_sub(mf, s_all, s_exc)
                        den = work.tile([P, 1], F32, tag="den")
                        nc.vector.tensor_sub(den, acc_all, acc_exc)
                        rec = work.tile([P, 1], F32, tag="rec")
                        nc.vector.reciprocal(rec, den)

                        if pending is not None:
                            combine_phase(*pending)
                        pending = (m, mf, rec)
                    combine_phase(*pending)

    nc.finalize()
    return nc


def _get_nc():
    global _CACHED
    if _CACHED is None:
        _CACHED = _build()
    return _CACHED


def _make_in_maps(e1, e2, rel_emb, W_fcs, b_fcs):
    e1 = np.asarray(e1, dtype=np.float32)
    e2 = np.asarray(e2, dtype=np.float32)
    rel_emb = np.asarray(rel_emb, dtype=np.float32)
    W_fcs = np.asarray(W_fcs, dtype=np.float32)
    b_fcs = np.asarray(b_fcs, dtype=np.float32).reshape(1, D)

    stackedT = np.ascontiguousarray(
        np.concatenate([e1, e2], axis=1).T)          # [2D, B]
    rel_T = np.ascontiguousarray(rel_emb.T)          # [2D, NR]
    W_T = np.ascontiguousarray(W_fcs.T)              # [2D, D]
    nsl = NLOC * P
    return [
        {
            "stackedT": np.ascontiguousarray(
                stackedT[:, c * BC:(c + 1) * BC]),
            "rel_T": rel_T,
            "rel_sliceT": np.ascontiguousarray(
                rel_emb[c * nsl:(c + 1) * nsl].T),
            "W_T": W_T,
            "b_fcs": b_fcs,
        }
        for c in range(N_CORES)
    ]


def kernel(e1, e2, rel_emb, W_fcs, b_fcs, **_ignored):
    nc = _get_nc()
    in_maps = _make_in_maps(e1, e2, rel_emb, W_fcs, b_fcs)
    res = run_bass_kernel_spmd(nc, in_maps, list(range(N_CORES)))
    return np.concatenate(
        [res.results[c]["out"] for c in range(N_CORES)], axis=0)
